# revision 17
# baseline (speedup 1.0000x reference)
"""Two-layer GRU encoder (B=64, T=12, N=325, D=2, H=256) on 8 TRN2 NeuronCores.

Data-parallel over batch (8 B-slices; per-core rows M = 8*325 = 2600, padded
to 2608 so every batch chunk is a multiple of 16). Feature-on-partition
layout: h lives as [128, 2, mw] tiles (dim1 = the two 128-feature halves).

PE: all K=256 matmuls (h @ Whzr, h0 @ Wx1) run as fp8-e4m3 DoubleRow
matmuls (2 MACs/cell/cycle: one MM per 128-feature gate chunk instead of
two). Weights are pre-scaled x16 into e4m3; the activation's scale=1/16
de-scales the PSUM for free. The candidate recurrent (rh @ Whh) stays bf16
so rh can stay bf16 (an fp8 operand would drop the DVE to 1x mode). The
layer-0 x-projection (combined layer-0 biases ride an ones-row) is zero-
padded from K=3 to K=65 so its tile_size rounds up to the full 128x128
array: sub-128 row-tile matmuls do NOT count as "PE busy" for the HAM clock
gate and would hold the whole kernel at the cold 1.2 GHz clock
(hardware-verified; tile_position packing is unusable for the same reason).

Scalar: layer 0 needs only three fused activations per chunk (sigmoid over
the 2-bank r tile, sigmoid over the 2-bank z tile, tanh over the 2-bank c
tile) because its biases rode the matmul. Layer 1 uses per-half activations
with the per-partition bias operand (6 acts) — cheaper than burning K=65
bias matmuls on the PE, which is nearly as loaded as the scalar engine.

Vector: rh = r*h and the blend h' = h + z*(c-h) in bf16 (2x_1p mode) plus
one bf16->fp8 tensor_copy per new h (2x_2p mode) to feed the DoubleRow MMs.
rh is emitted mid-blend of the previous chunk so it never head-blocks the
in-order DVE queue while its sigmoid is still running.

PSUM: three 2-bank tiles per chunk-layer — r (bufs=1), z (bufs=2), cand
(bufs=1) = all 8 banks. Splitting r/z/cand decouples the PE's next-chunk
matmuls from the scalar queue (the old 4-bank double-buffered layout gated
every chunk's matmuls on the previous chunk's tanh, costing ~1.3us/stage),
and sig_r fires before the z matmuls finish, shortening the sigmoid->rh->
candidate-matmul critical chain.
"""

import numpy as np
import ml_dtypes
from contextlib import ExitStack

import concourse.bass as bass
import concourse.tile as tile
from concourse import bacc, mybir
from concourse import bass_utils

BF16 = ml_dtypes.bfloat16
FP8 = ml_dtypes.float8_e4m3          # TRN FP8_EXP4 (max 240)
AF = mybir.ActivationFunctionType
PM = mybir.MatmulPerfMode
DT = mybir.dt

H = 256
T = 12
B = 64
N = 325
D = 2
NCORES = 8
B_SH = B // NCORES            # 8
M = B_SH * N                  # 2600
MP = 2608                     # padded so chunk widths are multiples of 16
_CWS = [448, 432, 432, 432, 432, 432]
CHUNKS = []
_o = 0
for _w in _CWS:
    CHUNKS.append((_o, _w))
    _o += _w
assert _o == MP
CWMAX = max(_CWS)
NCH = len(CHUNKS)
WS = 16.0                     # weight pre-scale (de-scaled in activations)

_CACHE = {}

# gate cols in weights: z [0:256], r [256:512], c [512:768]
# psum slot order, z/r phase: [ra, rb, za, zb]; cand phase: [ca, cb]
ZR_COLS = (256, 384, 0, 128)          # weight col starts for slots 0..3
C_COLS = (512, 640)
ZR_BCOL = (2, 3, 0, 1)                # bias1 cols (bz_a,bz_b,br_a,br_b,...)
C_BCOL = (4, 5)


def _build_nc():
    nc = bacc.Bacc("TRN2", target_bir_lowering=False, debug=False,
                   enable_asserts=False)
    bf = DT.bfloat16
    f8 = DT.float8e4
    f32 = DT.float32

    xt_d = nc.dram_tensor("xt", (65, T * MP), bf, kind="ExternalInput").ap()
    smallw_d = nc.dram_tensor("smallw", (65, 768), bf, kind="ExternalInput").ap()
    bias1_d = nc.dram_tensor("bias1", (128, 6), f32, kind="ExternalInput").ap()
    whzr0_d = nc.dram_tensor("whzr0q", (128, 1024), f8, kind="ExternalInput").ap()
    whh0_d = nc.dram_tensor("whh0", (128, 512), bf, kind="ExternalInput").ap()
    wx1_d = nc.dram_tensor("wx1q", (128, 1536), f8, kind="ExternalInput").ap()
    whzr1_d = nc.dram_tensor("whzr1q", (128, 1024), f8, kind="ExternalInput").ap()
    whh1_d = nc.dram_tensor("whh1", (128, 512), bf, kind="ExternalInput").ap()
    out_d = nc.dram_tensor("out", (2, NCH, 128, 2 * CWMAX), bf,
                           kind="ExternalOutput").ap()

    with tile.TileContext(nc) as tc, ExitStack() as ctx:
        const = ctx.enter_context(tc.tile_pool(name="const", bufs=1))
        hpool = ctx.enter_context(tc.tile_pool(name="hstate", bufs=1))
        work = ctx.enter_context(tc.tile_pool(name="work", bufs=4))
        psum = ctx.enter_context(tc.tile_pool(name="psum", bufs=2, space="PSUM"))

        # weights first (small, needed by t=0 compute), then xt split per
        # timestep so t=0 compute starts behind a 1/12-sized slice and the
        # rest streams under compute.
        # DMA order tracks t=0's needs: smallw + xt slice 0 + wx1q/bias1
        # unblock the first compute; recurrent weights are not needed until
        # t=1 and stream behind it.
        smallw = const.tile([65, 768], bf, tag="smallw", name="smallw")
        nc.sync.dma_start(smallw[:], smallw_d[:])
        xt_sb = const.tile([65, T * MP], bf, tag="xt", name="xt")
        for m0_, mw_ in CHUNKS:          # t=0 per chunk: first MM waits ~58KB
            nc.sync.dma_start(xt_sb[:, m0_:m0_ + mw_], xt_d[:, m0_:m0_ + mw_])

        def loadw(name, dram, shape, dtype):
            t_ = const.tile(list(shape), dtype, tag=name, name=name)
            nc.sync.dma_start(t_[:], dram[:])
            return t_

        wx1q = loadw("wx1q", wx1_d, (128, 2, 768), f8)
        bias1 = const.tile([128, 6], f32, tag="bias1", name="bias1")
        nc.sync.dma_start(bias1[:], bias1_d[:])
        whzr0q = loadw("whzr0q", whzr0_d, (128, 2, 512), f8)
        whh0 = loadw("whh0", whh0_d, (128, 512), bf)
        whzr1q = loadw("whzr1q", whzr1_d, (128, 2, 512), f8)
        whh1 = loadw("whh1", whh1_d, (128, 512), bf)
        for t_ in range(1, T):
            nc.sync.dma_start(xt_sb[:, t_ * MP:(t_ + 1) * MP],
                              xt_d[:, t_ * MP:(t_ + 1) * MP])

        hbf = {}
        hq = {}
        for L in (0, 1):
            for ci, (m0, mw) in enumerate(CHUNKS):
                for pp in (0, 1):
                    nm = f"h{L}_{ci}_{pp}"
                    hbf[(L, ci, pp)] = hpool.tile([128, 2, mw], bf,
                                                  tag="b" + nm, name="b" + nm)
                    hq[(L, ci, pp)] = hpool.tile([128, 2, mw], f8,
                                                 tag="q" + nm, name="q" + nm)

        def xp0_mm(out_ap, lcol0, t, m0, mw, start, stop):
            nc.tensor.matmul(
                out_ap,
                smallw[0:65, lcol0:lcol0 + 128],
                xt_sb[0:65, t * MP + m0: t * MP + m0 + mw],
                start=start, stop=stop, skip_group_check=True,
            )

        def dr_mm(out_ap, wtile, col0, rhs_ap, start, stop):
            nc.tensor.matmul(
                out_ap, wtile[:, :, col0:col0 + 128], rhs_ap,
                start=start, stop=stop, perf_mode=PM.DoubleRow,
                skip_group_check=True,
            )

        def zr_group(L, t, ci, P, half, lcol):
            first = t == 0
            m0, mw = CHUNKS[ci]
            pp_r = 1 - t % 2
            h_self = hq[(L, ci, pp_r)]      # recurrent input (fp8)
            if L == 0:
                xp0_mm(P[:, half, 0:mw], lcol, t, m0, mw,
                       start=True, stop=first)
                if not first:
                    dr_mm(P[:, half, 0:mw], whzr0q, lcol, h_self[:],
                          start=False, stop=True)
            else:
                dr_mm(P[:, half, 0:mw], wx1q, lcol,
                      hq[(0, ci, t % 2)][:], start=True, stop=first)
                if not first:
                    dr_mm(P[:, half, 0:mw], whzr1q, lcol, h_self[:],
                          start=False, stop=True)

        def stage_ar(L, t, ci, uid):
            """r-gate MMs + sig_r; hoisted one iteration ahead of the z
            phase so sig_r (and then rh) clear the in-order queues early."""
            mw = CHUNKS[ci][1]
            Pr = psum.tile([128, 2, 512], f32, tag="pr", bufs=1, name=f"Pr{uid}")
            for half in (0, 1):
                zr_group(L, t, ci, Pr, half, ZR_COLS[half])
            s_r = work.tile([128, 2, mw], bf, tag="sr", name=f"sr{uid}")
            if L == 0:
                nc.scalar.activation(s_r[:], Pr[:, :, 0:mw], AF.Sigmoid,
                                     scale=1.0 / WS)
            else:
                for s in (0, 1):
                    nc.scalar.activation(s_r[:, s, :], Pr[:, s, 0:mw],
                                         AF.Sigmoid, scale=1.0 / WS,
                                         bias=bias1[:, ZR_BCOL[s]:ZR_BCOL[s] + 1])
            return s_r

        def stage_az(L, t, ci, uid):
            mw = CHUNKS[ci][1]
            Pz = psum.tile([128, 2, 512], f32, tag="pz", bufs=2, name=f"Pz{uid}")
            for half in (0, 1):
                zr_group(L, t, ci, Pz, half, ZR_COLS[2 + half])
            s_z = work.tile([128, 2, mw], bf, tag="sz", name=f"sz{uid}")
            if L == 0:
                nc.scalar.activation(s_z[:], Pz[:, :, 0:mw], AF.Sigmoid,
                                     scale=1.0 / WS)
            else:
                for s in (0, 1):
                    nc.scalar.activation(s_z[:, s, :], Pz[:, s, 0:mw],
                                         AF.Sigmoid, scale=1.0 / WS,
                                         bias=bias1[:, ZR_BCOL[2 + s]:ZR_BCOL[2 + s] + 1])
            return s_z

        def emit_rh(L, t, ci, uid, s_r):
            # emitted late (after the previous chunk's blends) so the DVE
            # queue is not head-blocked waiting on sig_r
            mw = CHUNKS[ci][1]
            pp_r = 1 - t % 2
            rh = work.tile([128, 2, mw], bf, tag="rh", name=f"rh{uid}")
            nc.vector.tensor_mul(rh[:], s_r[:], hbf[(L, ci, pp_r)][:])
            return rh

        def stage_b(L, t, ci, uid, s_z, rh, rh_cb=None):
            """candidate phase (own 2-bank tile), tanh, blend, fp8 cast."""
            first = t == 0
            m0, mw = CHUNKS[ci]
            pp_w = t % 2
            pp_r = 1 - pp_w
            whh = whh0 if L == 0 else whh1
            Pc = psum.tile([128, 2, 512], f32, tag="pc", bufs=1, name=f"Pc{uid}")
            for half in (0, 1):
                lcol = C_COLS[half]
                if L == 0:
                    xp0_mm(Pc[:, half, 0:mw], lcol, t, m0, mw,
                           start=True, stop=first)
                else:
                    dr_mm(Pc[:, half, 0:mw], wx1q, lcol,
                          hq[(0, ci, t % 2)][:], start=True, stop=first)
                if not first:
                    for k in (0, 1):
                        nc.tensor.matmul(
                            Pc[:, half, 0:mw],
                            whh[:, k * 256 + half * 128: k * 256 + (half + 1) * 128],
                            rh[:, k, :],
                            start=False, stop=(k == 1), skip_group_check=True)
            c = work.tile([128, 2, mw], bf, tag="c", name=f"c{uid}")
            if L == 0:
                nc.scalar.activation(c[:], Pc[:, :, 0:mw], AF.Tanh,
                                     scale=1.0 / WS)
            else:
                for half in (0, 1):
                    nc.scalar.activation(c[:, half, :], Pc[:, half, 0:mw],
                                         AF.Tanh, scale=1.0 / WS,
                                         bias=bias1[:, C_BCOL[half]:C_BCOL[half] + 1])

            h_old = hbf[(L, ci, pp_r)]
            h_new = hbf[(L, ci, pp_w)]
            if first:
                nc.vector.tensor_mul(h_new[:], s_z[:], c[:])
            else:
                d = work.tile([128, 2, mw], bf, tag="d", name=f"d{uid}")
                nc.vector.tensor_sub(d[:], c[:], h_old[:])
                zd = work.tile([128, 2, mw], bf, tag="zd", name=f"zd{uid}")
                nc.vector.tensor_mul(zd[:], s_z[:], d[:])
                if rh_cb is not None:
                    rh_cb()          # next chunk's rh, mid-blend
                nc.vector.tensor_add(h_new[:], h_old[:], zd[:])
            if t == T - 1:
                # last step: ship the final hidden state out instead of
                # casting it for a next step that doesn't exist (L1) --
                # L0 still needs the cast for L1's x-projection this step.
                if L == 0:
                    nc.vector.tensor_copy(hq[(L, ci, pp_w)][:], h_new[:])
                nc.sync.dma_start(out_d[L, ci, :, 0:2 * mw], h_new[:])
            else:
                nc.vector.tensor_copy(hq[(L, ci, pp_w)][:], h_new[:])

        for t in range(T):
            for L in (0, 1):
                srs = {}
                szs = {}
                rhp = {}
                srs[0] = stage_ar(L, t, 0, f"L{L}t{t}c0")
                for i in range(NCH + 1):
                    if i + 1 < NCH:
                        srs[i + 1] = stage_ar(L, t, i + 1, f"L{L}t{t}c{i + 1}")
                    if i < NCH:
                        szs[i] = stage_az(L, t, i, f"L{L}t{t}c{i}")
                        if t > 0 and i == 0:
                            rhp[0] = emit_rh(L, t, 0, f"L{L}t{t}c0", srs[0])
                    if i >= 1:
                        cj = i - 1
                        uid = f"L{L}t{t}c{cj}"
                        rh_cb = None
                        if t > 0 and cj + 1 < NCH:
                            def rh_cb(L=L, t=t, cj=cj, sr=srs[cj + 1]):
                                rhp[cj + 1] = emit_rh(L, t, cj + 1,
                                                      f"L{L}t{t}c{cj + 1}", sr)
                        stage_b(L, t, cj, uid, szs.pop(cj), rhp.pop(cj, None),
                                rh_cb)



    nc.compile()
    return nc


def _prep_weights(inputs):
    def bf(x):
        return np.ascontiguousarray(np.asarray(x, np.float32), dtype=BF16)

    def q8(x):  # scale x16, clip to TRN e4m3 range, quantize
        y = np.clip(np.asarray(x, np.float32) * WS, -240, 240)
        return np.ascontiguousarray(y.astype(FP8))

    def kpack(w):  # (256, F) -> (128, 2, F): dim1 = K-halves
        w = np.asarray(w, np.float32)
        return np.stack([w[:128], w[128:]], axis=1)

    def kstack_bf(w):  # (256, C) -> (128, 2*C) bf16 halves side by side
        w = np.asarray(w, np.float32) * WS
        return bf(np.concatenate([w[:128], w[128:]], axis=1))

    ball = {}
    for L_, (bx, bhzr, bhh) in enumerate(
            [(inputs["bx0"], inputs["bhzr0"], inputs["bhh0"]),
             (inputs["bx1"], inputs["bhzr1"], inputs["bhh1"])]):
        bz = bx[:H] + bhzr[:H]
        br = bx[H:2 * H] + bhzr[H:2 * H]
        bc = bx[2 * H:] + bhh
        ball[L_] = np.concatenate([bz, br, bc])

    # smallw rows: [b0 combined (ones row); Wx0 row 0; Wx0 row 1], all x16.
    # Padded to 65 K-rows of zeros so the matmul tile_size rounds up to the
    # full 128x128 array — 32-row-tile matmuls don't count as "PE busy" for
    # the HAM clock gate and would hold the PE at 1.2 GHz.
    smallw = np.zeros((65, 768), np.float32)
    smallw[0] = ball[0] * WS
    smallw[1:3] = np.asarray(inputs["Wx0"], np.float32) * WS

    # bias1 (fp32, true units): cols = bz_a, bz_b, br_a, br_b, bc_a, bc_b
    b1 = ball[1]
    bias1 = np.stack([b1[0:128], b1[128:256], b1[256:384], b1[384:512],
                      b1[512:640], b1[640:768]], axis=1).astype(np.float32)

    return {
        "smallw": bf(smallw),
        "bias1": np.ascontiguousarray(bias1),
        "whzr0q": q8(kpack(inputs["Whzr0"])).reshape(128, 1024),
        "whh0": kstack_bf(inputs["Whh0"]),
        "wx1q": q8(kpack(inputs["Wx1"])).reshape(128, 1536),
        "whzr1q": q8(kpack(inputs["Whzr1"])).reshape(128, 1024),
        "whh1": kstack_bf(inputs["Whh1"]),
    }


def kernel(**inputs):
    X = np.asarray(inputs["X"], np.float32)
    shared = _prep_weights(inputs)

    if "nc" not in _CACHE:
        _CACHE["nc"] = _build_nc()
    nc = _CACHE["nc"]

    in_maps = []
    for c in range(NCORES):
        Xc = X[c * B_SH:(c + 1) * B_SH]                      # (8, T, N, D)
        xt = np.ascontiguousarray(Xc.transpose(3, 1, 0, 2)).reshape(D, T, M)
        buf = np.zeros((65, T, MP), np.float32)
        buf[0] = 1.0                                          # ones row
        buf[1:3, :, :M] = xt
        m = dict(shared)
        m["xt"] = np.ascontiguousarray(buf.reshape(65, T * MP), dtype=BF16)
        in_maps.append(m)
    _CACHE["in_maps"] = in_maps

    res = bass_utils.run_bass_kernel_spmd(nc, in_maps, core_ids=list(range(NCORES)))

    out = np.empty((2, B, N, H), np.float32)
    for c in range(NCORES):
        arr = np.asarray(res.results[c]["out"], dtype=np.float32)
        per_core = np.empty((2, M, H), np.float32)
        for ci, (m0, mw) in enumerate(CHUNKS):
            take = min(mw, M - m0)
            if take <= 0:
                continue
            blk = arr[:, ci, :, :2 * mw].reshape(2, 128, 2, mw)[..., :take]
            # [l, p, k, j] -> feature k*128+p, row m0+j
            per_core[:, m0:m0 + take, :] = blk.transpose(0, 3, 2, 1).reshape(2, take, H)
        out[:, c * B_SH:(c + 1) * B_SH] = per_core.reshape(2, B_SH, N, H)
    return out
